# revision 23
# baseline (speedup 1.0000x reference)
"""Trainium2 Bass kernel for one pre-LN transformer block (B=4, T=1024, C=1024,
H=16 heads, FF=4096), distributed over 8 NeuronCores with no collectives.

Sharding: core = (batch b, query-parity j). Each core computes K/V for all 1024
tokens of its batch but attention/FFN only for its 512 queries (tokens t with
t % 2 == j). Interleaved queries make the causal-mask tile structure identical
on every core (SPMD-safe). The host only permutes/transposes inputs and
re-interleaves the outputs.

v3: all PE operands are bf16. Q/K/V projections are computed PER HEAD-PAIR and
software-pipelined with the attention math at key-block granularity: each loop
body emits scores(hp+1, kb) and AV(hp, kb) back to back, so every PE matmul's
inputs were produced one pair-period earlier and the PE never stalls on the
current pair's exp -- keeping the HAM clock gate warm (an idle PE re-throttles
2.4->1.2 GHz after ~3.4us). The causal mask multiply runs on DVE in bf16 (2x
mode); LayerNorm's per-token scale/shift is applied via PE rank-1 outer
products (G0 = g (x) rstd, G1 = b (x) 1 + g (x) (-mu*rstd)) and two DVE ops;
head packing into 128-partition tiles uses partition-shifted DVE writes.
"""

import math
import sys
from dataclasses import dataclass

if "/opt/trn_rl_repo" not in sys.path:
    sys.path.insert(0, "/opt/trn_rl_repo")

import numpy as np


@dataclass(frozen=True)
class Cfg:
    B: int = 4
    T: int = 1024
    C: int = 1024
    H: int = 16
    FF: int = 4096

    @property
    def HD(self):
        return self.C // self.H

    @property
    def TQ(self):  # queries per core
        return self.T // 2

    @property
    def NCI(self):  # C / 128 feature tiles
        return self.C // 128

    @property
    def NFF(self):  # FF / 128 hidden tiles
        return self.FF // 128

    @property
    def NKB(self):  # key blocks of 128
        return self.T // 128

    @property
    def BW(self):  # token block width for LN1 phases
        return min(512, self.T)

    @property
    def NTB(self):  # token blocks over all T tokens
        return self.T // self.BW

    def s_kb(self, kb: int) -> int:
        """Start query-column of the computed score region for key block kb.
        Key blocks 0..NKB/2-1 hold this core's own-parity tokens in order;
        NKB/2.. hold the complementary-parity tokens. Causality allows the
        exact suffix [128*kbp, TQ) of queries per block."""
        return 128 * (kb % (self.NKB // 2))

    @property
    def pt_offs(self):
        """Column offsets of each key block's packed score region."""
        offs, o = [], 0
        for kb in range(self.NKB):
            offs.append(o)
            o += self.TQ - self.s_kb(kb)
        return offs + [o]


def build_nc(cfg: Cfg, n_cores: int = 8):
    import concourse.tile as tile
    from concourse import bacc, mybir

    f32 = mybir.dt.float32
    f32r = mybir.dt.float32r
    bf16 = mybir.dt.bfloat16
    Act = mybir.ActivationFunctionType
    Alu = mybir.AluOpType

    C, H, HD, FF = cfg.C, cfg.H, cfg.HD, cfg.FF
    NCI, NFF, NKB, NTB = cfg.NCI, cfg.NFF, cfg.NKB, cfg.NTB
    TQ, T = cfg.TQ, cfg.T
    NP = H // 2  # head pairs
    scale = 1.0 / math.sqrt(HD)
    offs = cfg.pt_offs

    nc = bacc.Bacc(
        "TRN2", target_bir_lowering=False, debug=False, num_devices=n_cores
    )

    # ---- DRAM I/O ----
    xpt = nc.dram_tensor("xpt", [C, T], f32r, kind="ExternalInput")
    msk = nc.dram_tensor("msk", [NKB, 128, 128], bf16, kind="ExternalInput")
    dscr_a = nc.dram_tensor("dscr_a", [H * TQ], bf16, kind="Internal")
    dscr_b = nc.dram_tensor("dscr_b", [H * TQ], bf16, kind="Internal")
    wq = nc.dram_tensor("wq", [C, C], bf16, kind="ExternalInput")
    wk = nc.dram_tensor("wk", [C, C], bf16, kind="ExternalInput")
    wv = nc.dram_tensor("wv", [C, C], bf16, kind="ExternalInput")
    wp = nc.dram_tensor("wp", [C, C], bf16, kind="ExternalInput")
    w1 = nc.dram_tensor("w1", [C, FF], bf16, kind="ExternalInput")
    w2 = nc.dram_tensor("w2", [FF, C], bf16, kind="ExternalInput")
    ln1g = nc.dram_tensor("ln1g", [C], f32r, kind="ExternalInput")
    ln1b = nc.dram_tensor("ln1b", [C], f32r, kind="ExternalInput")
    ln2g = nc.dram_tensor("ln2g", [C], f32r, kind="ExternalInput")
    ln2b = nc.dram_tensor("ln2b", [C], f32r, kind="ExternalInput")
    bpj = nc.dram_tensor("bpj", [C], f32, kind="ExternalInput")
    b1 = nc.dram_tensor("b1", [FF], f32, kind="ExternalInput")
    b2 = nc.dram_tensor("b2", [C], f32, kind="ExternalInput")
    yt = nc.dram_tensor("yt", [C, TQ], f32, kind="ExternalOutput")

    with (
        nc.allow_low_precision(reason="bf16 matmul operands"),
        tile.TileContext(nc) as tc,
    ):
        # ---------------- x DMA first (LN1 starts as soon as possible) ----
        raw, free_raw = tc.tile([128, NCI, T], f32r, name="raw", side="right")
        xpt_r = xpt.rearrange("(ci p) t -> ci p t", p=128)
        for half in range(NTB):
            hsl = slice(half * cfg.BW, (half + 1) * cfg.BW)
            for ci in range(NCI):
                nc.sync.dma_start(out=raw[:, ci, hsl], in_=xpt_r[ci][:, hsl])

        # ---------------- persistent constants / params ----------------
        onesf, free_onesf = tc.tile([128, 512], f32, name="onesf")
        nc.vector.memset(onesf, 1.0)
        ones128, free_ones128 = tc.tile([128, 1], f32r, name="ones128")
        nc.vector.tensor_copy(out=ones128, in_=onesf[:, 0:1])
        ones128b, free_ones128b = tc.tile([128, 1], bf16, name="ones128b")
        nc.vector.tensor_copy(out=ones128b, in_=onesf[:, 0:1])
        onesw, free_onesw = tc.tile([1, cfg.BW], f32r, name="onesw")
        nc.vector.tensor_copy(out=onesw, in_=onesf[0:1, 0 : cfg.BW])
        # lhsT row of ones at partition 64 for the per-head recip broadcast
        oneshi, free_oneshi = tc.tile([65, HD], bf16, name="oneshi")
        nc.vector.tensor_copy(out=oneshi, in_=onesf[0:65, 0:HD])
        epst, free_epst = tc.tile([1, 1], f32, name="epst")
        nc.vector.memset(epst, 1e-5)

        # LN gamma/beta as [1, C] rows (lhsT of the rank-1 broadcasts)
        g1r, free_g1r = tc.tile([1, C], f32r, name="g1r")
        b1r, free_b1r = tc.tile([1, C], f32r, name="b1r")
        g2r, free_g2r = tc.tile([1, C], f32r, name="g2r")
        b2r, free_b2r = tc.tile([1, C], f32r, name="b2r")
        for ptile, v in ((g1r, ln1g), (b1r, ln1b), (g2r, ln2g), (b2r, ln2b)):
            nc.sync.dma_start(
                out=ptile, in_=v.rearrange("(o a) -> o a", o=1)
            )
        bpjt, free_bpjt = tc.tile([128, NCI], f32, name="bpjt")
        nc.sync.dma_start(out=bpjt, in_=bpj.rearrange("(a p) -> p a", p=128))
        b1t, free_b1t = tc.tile([128, NFF], f32, name="b1t")
        nc.sync.dma_start(out=b1t, in_=b1.rearrange("(a p) -> p a", p=128))
        b2t, free_b2t = tc.tile([128, NCI], f32, name="b2t")
        nc.sync.dma_start(out=b2t, in_=b2.rearrange("(a p) -> p a", p=128))
        mskt, free_mskt = tc.tile([128, NKB, 128], bf16, name="mskt")
        nc.sync.dma_start(out=mskt, in_=msk.rearrange("k p m -> p k m"))

        # one PSUM pool for the whole kernel. PSUM is 8 banks of [128, 2KB]:
        # tag "mm" gets 6 rotating slots, tag "av" the other 2 (the AV
        # accumulators live across a whole pair body, so they must not sit
        # in the "mm" rotation or the scores allocs would deadlock on them).
        ps_all = tc.alloc_tile_pool(name="ps_all", bufs=6, space="PSUM")
        wstream = tc.alloc_tile_pool(name="wstream", bufs=8)
        wpair = tc.alloc_tile_pool(name="wpair", bufs=3)

        fill_i = [0]

        def emit_fill(n=2):
            """Dependency-free PE matmuls: keep the HAM clock gate open
            through stretches where real PE work is sparse."""
            with nc.named_scope("fill"):
                pw = ps_all.tile([128, 512], f32, tag="mm", name=f"fw{fill_i[0]}")
                fill_i[0] += 1
                for r in range(n):
                    nc.tensor.matmul(
                        pw, onesf[:, 0:128].bitcast(f32r), onesf.bitcast(f32r),
                        start=(r == 0), stop=(r == n - 1),
                    )

        with nc.named_scope("warmup"):
            for wu in range(6):
                emit_fill(2)

        # x2T = x + attnproj (residual 1), written in the proj phase
        x2t, free_x2t = tc.tile([128, NCI, TQ], bf16, name="x2t")
        # packed normalized heads [128, pair, TQ]
        att2, free_att2 = tc.tile([128, NP, TQ], bf16, name="att2")

        def layernorm(src_ap_fn, dst, g_row, b_row, n_blocks, blk_w, scopename,
                      ones_st=ones128, post_block=None, fills=False):
            """src_ap_fn(ci, sl) -> [128, blk_w] f32r AP; dst [128, NCI, *]
            bf16. Stats via ones-vector matmuls. Apply: per (block, ci) the
            affine per-token transform is built as two PE rank-1 outer
            products G0 = g (x) rstd and G1 = b (x) 1 + g (x) (-mu*rstd),
            then dst = src*G0 + G1 in two DVE ops."""
            with (
                nc.named_scope(scopename),
                tc.tile_pool(name=f"{scopename}_sb", bufs=max(3, n_blocks + 1)) as sbp,
            ):
                stats = []
                for tb in range(n_blocks):
                    sl = slice(tb * blk_w, (tb + 1) * blk_w)
                    psx = ps_all.tile([1, blk_w], f32, tag="mm", name=f"psx{tb}")
                    psq = ps_all.tile([1, blk_w], f32, tag="mm", name=f"psq{tb}")
                    for ci in range(NCI):
                        nc.tensor.matmul(
                            psx, ones_st, src_ap_fn(ci, sl),
                            start=(ci == 0), stop=(ci == NCI - 1),
                        )
                    for ci in range(NCI):
                        x_ap = src_ap_fn(ci, sl)
                        sq = sbp.tile([128, blk_w], bf16, tag="sq", name=f"sq{tb}_{ci}")
                        if ci % 2 == 0:
                            nc.scalar.activation(out=sq, in_=x_ap, func=Act.Square)
                        else:
                            nc.vector.tensor_mul(out=sq, in0=x_ap, in1=x_ap)
                        nc.tensor.matmul(
                            psq, ones128b, sq,
                            start=(ci == 0), stop=(ci == NCI - 1),
                        )
                    stats.append((psx, psq))
                    if fills:
                        emit_fill(2)
                rows = []
                for tb in range(n_blocks):
                    psx, psq = stats[tb]
                    mu = sbp.tile([1, blk_w], f32r, tag="rs", bufs=5, name=f"mu{tb}")
                    nc.scalar.mul(out=mu, in_=psx, mul=1.0 / C)
                    nmu = sbp.tile([1, blk_w], f32r, tag="rs", bufs=5, name=f"nmu{tb}")
                    nc.scalar.mul(out=nmu, in_=mu, mul=-1.0)
                    ms = sbp.tile([1, blk_w], f32r, tag="rs", bufs=5, name=f"ms{tb}")
                    nc.scalar.mul(out=ms, in_=psq, mul=1.0 / C)
                    mu2 = sbp.tile([1, blk_w], f32r, tag="rs", bufs=5, name=f"mu2{tb}")
                    nc.scalar.activation(out=mu2, in_=mu, func=Act.Square)
                    var = sbp.tile([1, blk_w], f32r, tag="rs", bufs=5, name=f"var{tb}")
                    nc.vector.tensor_sub(out=var, in0=ms, in1=mu2)
                    # rstd = exp(-0.5*ln(var+eps)): two fast ACT row ops
                    sd = sbp.tile([1, blk_w], f32r, tag="rs", bufs=5, name=f"sd{tb}")
                    nc.scalar.activation(
                        out=sd, in_=var, func=Act.Ln, bias=epst
                    )
                    c0 = sbp.tile([1, blk_w], f32r, tag=f"c0_{tb}", bufs=1)
                    nc.scalar.activation(
                        out=c0, in_=sd, func=Act.Exp, scale=-0.5
                    )
                    c1 = sbp.tile([1, blk_w], f32r, tag=f"c1_{tb}", bufs=1)
                    nc.vector.tensor_mul(out=c1, in0=nmu, in1=c0)
                    rows.append((c0, c1))
                    if fills:
                        emit_fill(2)
                for tb in range(n_blocks):
                    sl = slice(tb * blk_w, (tb + 1) * blk_w)
                    c0, c1 = rows[tb]
                    for ci in range(NCI):
                        gsl = slice(128 * ci, 128 * (ci + 1))
                        x_ap = src_ap_fn(ci, sl)
                        G0 = ps_all.tile([128, blk_w], f32, tag="mm", name=f"G0_{tb}_{ci}")
                        nc.tensor.matmul(G0, g_row[:, gsl], c0)
                        G1 = ps_all.tile([128, blk_w], f32, tag="mm", name=f"G1_{tb}_{ci}")
                        nc.tensor.matmul(
                            G1, b_row[:, gsl], onesw[:, 0:blk_w],
                            start=True, stop=False,
                        )
                        nc.tensor.matmul(
                            G1, g_row[:, gsl], c1, start=False, stop=True
                        )
                        tmp = sbp.tile([128, blk_w], bf16, tag="tmp", name=f"t{tb}_{ci}")
                        nc.vector.tensor_mul(out=tmp, in0=x_ap, in1=G0)
                        nc.vector.tensor_add(out=dst[:, ci, sl], in0=tmp, in1=G1)
                        if fills and ci % 3 == 2:
                            emit_fill(1)
                    if post_block is not None:
                        post_block(tb)

        # ---------------- attention: per-pair QKV pipelined with softmax ---
        a1, free_a1 = tc.tile([128, NCI, T], bf16, name="a1", side="right")
        # vt: per key block, per head: 64 v-columns + a ones column (fused
        # softmax denominator row in the AV matmul output).
        vt, free_vt = tc.tile([128, NKB, H, HD + 1], bf16, name="vt", side="right")
        for kb in range(NKB):
            nc.vector.tensor_copy(
                out=vt[:, kb, :, HD : HD + 1], in_=onesf[:, 0:H].unsqueeze(2)
            )
        # att holds, per head, O^T rows 0..HD-1 (unnormalized) and the
        # reciprocal softmax denominator in row 64.
        att, free_att = tc.tile([65, H, TQ], bf16, name="att", side="right")

        wq_r = wq.rearrange("(ci p) c -> p ci c", p=128)
        wk_r = wk.rearrange("(ci p) c -> p ci c", p=128)
        wv_r = wv.rearrange("(ci p) c -> p ci c", p=128)

        qk_pool = tc.alloc_tile_pool(name="qk_pool", bufs=3, side="right")
        pt_pool = tc.alloc_tile_pool(name="pt_pool", bufs=4, side="right")

        qts, kts, pts, avps, wvts = {}, {}, {}, {}, {}

        wqts, wkts = {}, {}

        def emit_qk_dma(hp):
            """Prefetch Q/K weight slices for head pair hp."""
            if hp >= NP:
                return
            csl = slice(128 * hp, 128 * (hp + 1))
            wqt = wpair.tile([128, NCI, 128], bf16, tag="wq", bufs=2, name=f"wq{hp}")
            nc.sync.dma_start(out=wqt, in_=wq_r[:, :, csl])
            wkt = wpair.tile([128, NCI, 128], bf16, tag="wk", bufs=2, name=f"wk{hp}")
            nc.sync.dma_start(out=wkt, in_=wk_r[:, :, csl])
            wqts[hp], wkts[hp] = wqt, wkt

        def emit_q(hp):
            """Q projection for head pair hp (feature rows 128*hp..)."""
            if hp >= NP:
                return
            with nc.named_scope("qkv"):
                qt = qk_pool.tile([128, TQ], bf16, tag="qt", name=f"qt{hp}")
                pq = ps_all.tile([128, TQ], f32, tag="mm", name=f"pq{hp}")
                for ci in range(NCI):
                    nc.tensor.matmul(
                        pq, wqts[hp][:, ci, :], a1[:, ci, 0:TQ],
                        start=(ci == 0), stop=(ci == NCI - 1),
                    )
                nc.vector.tensor_copy(out=qt, in_=pq)
                qts[hp] = qt

        def emit_k(hp, tb):
            """K projection for head pair hp, token half tb."""
            if hp >= NP:
                return
            with nc.named_scope("qkv"):
                if tb == 0:
                    kts[hp] = qk_pool.tile([128, T], bf16, tag="kt", name=f"kt{hp}")
                sl = slice(512 * tb, 512 * (tb + 1))
                pk = ps_all.tile([128, 512], f32, tag="mm", name=f"pk{hp}_{tb}")
                for ci in range(NCI):
                    nc.tensor.matmul(
                        pk, wkts[hp][:, ci, :], a1[:, ci, sl],
                        start=(ci == 0), stop=(ci == NCI - 1),
                    )
                nc.vector.tensor_copy(out=kts[hp][:, sl], in_=pk)

        def emit_vdma(g):
            """Prefetch the V weight slice for heads 4g..4g+3."""
            if g >= H // 4:
                return
            csl = slice(256 * g, 256 * (g + 1))
            wvt = wpair.tile([128, NCI, 256], bf16, tag="wv", bufs=2, name=f"wv{g}")
            nc.sync.dma_start(out=wvt, in_=wv_r[:, :, csl])
            wvts[g] = wvt

        def emit_vchunk_kb(g, kb):
            """V projection for heads 4g..4g+3 (pairs 2g, 2g+1), one key
            block. Activations stationary, 256 weight columns moving."""
            if g >= H // 4:
                return
            with nc.named_scope("qkv"):
                kbsl = slice(128 * kb, 128 * (kb + 1))
                pv = ps_all.tile([128, 256], f32, tag="mm", name=f"pv{g}_{kb}")
                for ci in range(NCI):
                    nc.tensor.matmul(
                        pv, a1[:, ci, kbsl], wvts[g][:, ci, :],
                        start=(ci == 0), stop=(ci == NCI - 1),
                    )
                nc.vector.tensor_copy(
                    out=vt[:, kb, 4 * g : 4 * g + 4, 0:HD],
                    in_=pv.rearrange("p (h d) -> p h d", h=4),
                )

        def emit_scores_kb(hp, kb):
            """Scores + exp + causal mask for both heads of pair hp, one
            key block."""
            if hp >= NP:
                return
            with nc.named_scope("attn"):
                if kb == 0:
                    p0 = pt_pool.tile([128, offs[-1]], bf16, tag="pt", name=f"pt{2 * hp}")
                    p1 = pt_pool.tile([128, offs[-1]], bf16, tag="pt", name=f"pt{2 * hp + 1}")
                    pts[hp] = (p0, p1)
                qt, kt = qts[hp], kts[hp]
                s = cfg.s_kb(kb)
                n = TQ - s
                kbsl = slice(128 * kb, 128 * (kb + 1))
                pss = []
                for idx in range(2):
                    po = idx * HD
                    ps_s = ps_all.tile([128, 512], f32, tag="mm", name=f"sc{hp}_{kb}_{idx}")
                    nc.tensor.matmul(
                        ps_s[:, 0:n],
                        kt[po : po + HD, kbsl],
                        qt[po : po + HD, s:TQ],
                    )
                    pss.append(ps_s)
                for idx in range(2):
                    dst = pts[hp][idx]
                    nc.scalar.activation(
                        out=dst[:, offs[kb] : offs[kb] + n],
                        in_=pss[idx][:, 0:n],
                        func=Act.Exp, scale=scale,
                    )
                    # causal mask: first 128 columns of the block's region
                    # are the partially-visible diagonal zone. bf16 -> DVE 2x.
                    nc.vector.tensor_mul(
                        out=dst[:, offs[kb] : offs[kb] + 128],
                        in0=dst[:, offs[kb] : offs[kb] + 128],
                        in1=mskt[:, kb, :],
                    )

        def emit_av_kb(hp, kb):
            """One key block of the AV accumulation for both heads of pair
            hp (inputs were produced one pair-period earlier)."""
            if hp < 0:
                return
            with nc.named_scope("attn"):
                s = cfg.s_kb(kb)
                for idx in range(2):
                    h = 2 * hp + idx
                    if kb == 0:
                        avps[h] = ps_all.tile(
                            [65, TQ], f32, tag="av", bufs=2, name=f"av{h}"
                        )
                    nc.tensor.matmul(
                        avps[h][:, s:TQ],
                        vt[:, kb, h, :],
                        pts[hp][idx][:, offs[kb] : offs[kb + 1]],
                        start=(kb == 0), stop=(kb == NKB - 1),
                        skip_group_check=True,
                    )

        def emit_av_finish(hp):
            """Copy unnormalized O^T and denominator rows out of PSUM."""
            with nc.named_scope("attn"):
                for idx in range(2):
                    h = 2 * hp + idx
                    if hp >= 6:
                        nc.scalar.copy(out=att[0:64, h, :], in_=avps[h][0:64, :])
                    else:
                        nc.vector.tensor_copy(
                            out=att[0:64, h, :], in_=avps[h][0:64, :]
                        )
                    nc.vector.tensor_copy(
                        out=att[64:65, h, :], in_=avps[h][64:65, :]
                    )

        def emit_recip(hs, nh):
            """Batch-reciprocal the denominator rows of heads hs..hs+nh-1
            via a DRAM round-trip spreading them over 128 partitions."""
            assert (nh * TQ) % 128 == 0
            with nc.named_scope("attn"):
                hsl = slice(hs, hs + nh)
                fl = nh * TQ // 128
                nc.sync.dma_start(
                    out=dscr_a.rearrange("(o h t) -> o h t", o=1, h=H)[:, hsl, :],
                    in_=att[64:65, hsl, :],
                )
                dwide = pt_pool.tile([128, fl], bf16, tag="dw", bufs=2, name=f"dw{hs}")
                nc.sync.dma_start(
                    out=dwide,
                    in_=dscr_a[hs * TQ : (hs + nh) * TQ].rearrange(
                        "(p f) -> p f", p=128
                    ),
                )
                nc.vector.reciprocal(out=dwide, in_=dwide)
                nc.sync.dma_start(
                    out=dscr_b[hs * TQ : (hs + nh) * TQ].rearrange(
                        "(p f) -> p f", p=128
                    ),
                    in_=dwide,
                )
                nc.sync.dma_start(
                    out=att[64:65, hsl, :],
                    in_=dscr_b.rearrange("(o h t) -> o h t", o=1, h=H)[:, hsl, :],
                )

        def emit_norm(hp):
            """Normalize pair hp's heads by the reciprocal denominators and
            pack them into att2[:, hp, :] (odd head via partition-shifted
            DVE write to partitions 64..127)."""
            with nc.named_scope("attn"):
                for idx in range(2):
                    h = 2 * hp + idx
                    bc = ps_all.tile([64, TQ], f32, tag="mm", name=f"bc{h}")
                    nc.tensor.matmul(bc, oneshi[64:65, :], att[64:65, h, :])
                    psl = slice(64 * idx, 64 * idx + 64)
                    nc.vector.tensor_mul(
                        out=att2[psl, hp, :], in0=att[0:64, h, :], in1=bc
                    )

        # pipeline: scores of pair hp+1, AV of pair hp, V of pair hp+1,
        # and Q/K of pair hp+2 are interleaved at key-block granularity, so
        # every PE matmul's inputs are one pair-period old and the PE
        # streams continuously (keeps the HAM clock gate warm).
        def ln1_post_block(tb):
            """Interleave the first pair's projections/scores with LN1's
            DVE-bound apply phase: block 0 gates queries + the first key
            half, block 1 the rest."""
            if tb == 0:
                emit_qk_dma(0)
                emit_qk_dma(1)
                emit_vdma(0)
                emit_q(0)
                emit_k(0, 0)
                for kb in range(NKB // 2):
                    emit_scores_kb(0, kb)
                    emit_vchunk_kb(0, kb)
            else:
                emit_k(0, 1)
                for kb in range(NKB // 2, NKB):
                    emit_scores_kb(0, kb)
                    emit_vchunk_kb(0, kb)
                emit_q(1)
                emit_k(1, 0)
                emit_k(1, 1)

        # ---------------- LN1 over all T tokens ----------------
        layernorm(
            lambda ci, sl: raw[:, ci, sl], a1, g1r, b1r, NTB, cfg.BW, "ln1",
            post_block=ln1_post_block, fills=True,
        )
        for hp in range(NP):
            g = (hp + 1) // 2
            vchunk = hp % 2 == 1 and g < H // 4
            if vchunk:
                emit_vdma(g)
            for kb in range(NKB):
                emit_scores_kb(hp + 1, kb)
                emit_av_kb(hp, kb)
                if vchunk:
                    emit_vchunk_kb(g, kb)
                if hp >= 6 and kb % 2 == 0:
                    emit_fill(2)
                if kb == 1:
                    emit_qk_dma(hp + 2)
                elif kb == 3:
                    emit_q(hp + 2)
                elif kb == 5:
                    emit_k(hp + 2, 0)
                elif kb == 7:
                    emit_k(hp + 2, 1)
            emit_av_finish(hp)
            emit_recip(2 * hp, 2)
            if hp >= 1:
                # previous pair's reciprocals are back from DRAM by now
                emit_norm(hp - 1)
        emit_fill(4)
        emit_norm(7)
        emit_fill(8)

        # ---------------- attention out-proj + residual 1 ----------------
        with nc.named_scope("proj"):
            pp = [
                ps_all.tile(
                    [128, TQ], f32,
                    tag=("mm" if i < 6 else "av"),
                    bufs=(6 if i < 6 else 2),
                    name=f"pp{i}",
                )
                for i in range(NCI)
            ]
            for ci in range(NCI):
                wt = wstream.tile([128, C], bf16, tag="w", name=f"wpt{ci}")
                nc.sync.dma_start(out=wt, in_=wp[128 * ci : 128 * (ci + 1)])
                for co in range(NCI):
                    nc.tensor.matmul(
                        pp[co],
                        wt[:, 128 * co : 128 * (co + 1)],
                        att2[:, ci, :],
                        start=(ci == 0), stop=(ci == NCI - 1),
                    )
            for co in range(NCI):
                nc.vector.scalar_tensor_tensor(
                    out=x2t[:, co, :],
                    in0=pp[co],
                    scalar=bpjt[:, co : co + 1],
                    in1=raw[:, co, 0:TQ],
                    op0=Alu.add,
                    op1=Alu.add,
                )
        emit_fill(6)
        pt_pool.release()
        qk_pool.release()
        free_att()
        free_vt()
        free_a1()
        free_raw()

        # ---------------- LN2 ----------------
        a2, free_a2 = tc.tile([128, NCI, TQ], bf16, name="a2", side="right")
        layernorm(
            lambda ci, sl: x2t[:, ci, sl], a2, g2r, b2r, 1, TQ, "ln2",
            ones_st=ones128b, fills=True,
        )

        # ---------------- FFN ----------------
        emit_fill(6)
        hsb, free_hsb = tc.tile([128, NFF, TQ], bf16, name="hsb", side="right")
        with nc.named_scope("ffn1"):
            for cog in range(NFF // 8):
                pf = [
                    ps_all.tile(
                        [128, TQ], f32,
                        tag=("mm" if i < 6 else "av"),
                        bufs=(6 if i < 6 else 2),
                        name=f"pf{cog}_{i}",
                    )
                    for i in range(8)
                ]
                for ci in range(NCI):
                    wt = wstream.tile([128, 1024], bf16, tag="w", name=f"w1t{cog}_{ci}")
                    nc.sync.dma_start(
                        out=wt,
                        in_=w1[
                            128 * ci : 128 * (ci + 1),
                            1024 * cog : 1024 * (cog + 1),
                        ],
                    )
                    for co in range(8):
                        nc.tensor.matmul(
                            pf[co],
                            wt[:, 128 * co : 128 * (co + 1)],
                            a2[:, ci, :],
                            start=(ci == 0), stop=(ci == NCI - 1),
                        )
                for co in range(8):
                    hco = cog * 8 + co
                    nc.scalar.activation(
                        out=hsb[:, hco, :],
                        in_=pf[co],
                        func=Act.Gelu,
                        bias=b1t[:, hco : hco + 1],
                    )

        yts, free_yts = tc.tile([128, NCI, TQ], f32, name="yts", side="right")
        with nc.named_scope("ffn2"):
            py = [
                ps_all.tile(
                    [128, TQ], f32,
                    tag=("mm" if i < 6 else "av"),
                    bufs=(6 if i < 6 else 2),
                    name=f"py{i}",
                )
                for i in range(NCI)
            ]
            for fi in range(NFF):
                wt = wstream.tile([128, C], bf16, tag="w", name=f"w2t{fi}")
                nc.sync.dma_start(out=wt, in_=w2[128 * fi : 128 * (fi + 1)])
                for co in range(NCI):
                    nc.tensor.matmul(
                        py[co],
                        wt[:, 128 * co : 128 * (co + 1)],
                        hsb[:, fi, :],
                        start=(fi == 0), stop=(fi == NFF - 1),
                    )
            yt_r = yt.rearrange("(ci p) t -> ci p t", p=128)
            for co in range(NCI):
                nc.vector.scalar_tensor_tensor(
                    out=yts[:, co, :],
                    in0=py[co],
                    scalar=b2t[:, co : co + 1],
                    in1=x2t[:, co, :],
                    op0=Alu.add,
                    op1=Alu.add,
                )
                nc.sync.dma_start(out=yt_r[co], in_=yts[:, co, :])
        free_yts()
        free_hsb()
        free_a2()
        free_att2()
        free_x2t()
        wpair.release()
        wstream.release()
        ps_all.release()
        free_mskt()
        free_b2t()
        free_b1t()
        free_bpjt()
        free_b2r()
        free_g2r()
        free_b1r()
        free_g1r()
        free_epst()
        free_oneshi()
        free_onesw()
        free_ones128b()
        free_ones128()
        free_onesf()

    nc.compile()
    return nc


def prep_core_inputs(cfg: Cfg, inputs: dict, b: int, j: int) -> dict:
    """Host-side slicing/permutation for core (batch b, parity j)."""
    T, TQ, NKB = cfg.T, cfg.TQ, cfg.NKB
    x = np.asarray(inputs["x"])
    perm = np.concatenate([np.arange(j, T, 2), np.arange(1 - j, T, 2)])
    xp = x[b][perm]  # [T, C]
    xpt = np.ascontiguousarray(xp.T, dtype=np.float32)

    import ml_dtypes

    qtok = perm[:TQ]
    ktok = perm
    mask = np.ones((NKB, 128, 128), dtype=np.float32)
    for kb in range(NKB):
        s = cfg.s_kb(kb)
        kt = ktok[128 * kb : 128 * (kb + 1)]  # [128]
        qt = qtok[s : s + 128]  # [128]
        allowed = qt[None, :] >= kt[:, None]  # [128, 128]
        mask[kb] = np.where(allowed, 1.0, 0.0)
    return {"xpt": xpt, "msk": mask.astype(ml_dtypes.bfloat16)}


def prep_shared_inputs(cfg: Cfg, inputs: dict) -> dict:
    import ml_dtypes

    C = cfg.C
    f32 = np.float32
    bf16 = ml_dtypes.bfloat16

    def wq2d(w):  # [H, C, HD] -> [C, H*HD]
        w = np.asarray(w)
        return np.ascontiguousarray(
            w.transpose(1, 0, 2).reshape(C, C)
        ).astype(bf16)

    return {
        "wq": wq2d(inputs["Wq"]),
        "wk": wq2d(inputs["Wk"]),
        "wv": wq2d(inputs["Wv"]),
        "wp": np.ascontiguousarray(inputs["Wproj"]).astype(bf16),
        "w1": np.ascontiguousarray(inputs["W1"]).astype(bf16),
        "w2": np.ascontiguousarray(inputs["W2"]).astype(bf16),
        "ln1g": np.ascontiguousarray(inputs["ln1_g"], dtype=f32),
        "ln1b": np.ascontiguousarray(inputs["ln1_b"], dtype=f32),
        "ln2g": np.ascontiguousarray(inputs["ln2_g"], dtype=f32),
        "ln2b": np.ascontiguousarray(inputs["ln2_b"], dtype=f32),
        "bpj": np.ascontiguousarray(inputs["bproj"], dtype=f32),
        "b1": np.ascontiguousarray(inputs["b1"], dtype=f32),
        "b2": np.ascontiguousarray(inputs["b2"], dtype=f32),
    }


def run(cfg: Cfg, inputs: dict, n_cores: int = 8, trace: bool = False):
    from concourse.bass_utils import run_bass_kernel_spmd

    nc = build_nc(cfg, n_cores=n_cores)
    shared = prep_shared_inputs(cfg, inputs)
    in_maps = []
    cores = []
    for core in range(n_cores):
        b, j = divmod(core, 2)
        b = b % cfg.B
        in_maps.append({**prep_core_inputs(cfg, inputs, b, j), **shared})
        cores.append((b, j))
    res = run_bass_kernel_spmd(
        nc, in_maps, core_ids=list(range(n_cores)), trace=trace
    )
    out = np.zeros((cfg.B, cfg.T, cfg.C), dtype=np.float32)
    for core, (b, j) in enumerate(cores):
        ytv = res.results[core]["yt"]  # [C, TQ]
        perm = np.concatenate(
            [np.arange(j, cfg.T, 2), np.arange(1 - j, cfg.T, 2)]
        )
        out[b, perm[: cfg.TQ], :] = ytv.T
    return out, res


def kernel(**inputs) -> np.ndarray:
    out, _ = run(Cfg(), inputs, n_cores=8, trace=False)
    return out


if __name__ == "__main__":
    # quick self-exercise at full size with random data
    rng = np.random.default_rng(0)
    cfg = Cfg()
    ins = {
        "x": rng.standard_normal((cfg.B, cfg.T, cfg.C)).astype(np.float32),
        "ln1_g": np.ones(cfg.C, np.float32),
        "ln1_b": np.zeros(cfg.C, np.float32),
        "ln2_g": np.ones(cfg.C, np.float32),
        "ln2_b": np.zeros(cfg.C, np.float32),
        "Wq": rng.standard_normal((cfg.H, cfg.C, cfg.HD)).astype(np.float32)
        * 0.02,
        "Wk": rng.standard_normal((cfg.H, cfg.C, cfg.HD)).astype(np.float32)
        * 0.02,
        "Wv": rng.standard_normal((cfg.H, cfg.C, cfg.HD)).astype(np.float32)
        * 0.02,
        "Wproj": rng.standard_normal((cfg.C, cfg.C)).astype(np.float32) * 0.02,
        "bproj": np.zeros(cfg.C, np.float32),
        "W1": rng.standard_normal((cfg.C, cfg.FF)).astype(np.float32) * 0.02,
        "b1": np.zeros(cfg.FF, np.float32),
        "W2": rng.standard_normal((cfg.FF, cfg.C)).astype(np.float32) * 0.02,
        "b2": np.zeros(cfg.C, np.float32),
    }
    y = kernel(**ins)
    print("ran, out", y.shape, y.dtype, float(np.abs(y).max()))


# revision 24
# speedup vs baseline: 1.0186x; 1.0186x over previous
"""Trainium2 Bass kernel for one pre-LN transformer block (B=4, T=1024, C=1024,
H=16 heads, FF=4096), distributed over 8 NeuronCores with no collectives.

Sharding: core = (batch b, query-parity j). Each core computes K/V for all 1024
tokens of its batch but attention/FFN only for its 512 queries (tokens t with
t % 2 == j). Interleaved queries make the causal-mask tile structure identical
on every core (SPMD-safe). The host only permutes/transposes inputs and
re-interleaves the outputs.

v3: all PE operands are bf16. Q/K/V projections are computed PER HEAD-PAIR and
software-pipelined with the attention math at key-block granularity: each loop
body emits scores(hp+1, kb) and AV(hp, kb) back to back, so every PE matmul's
inputs were produced one pair-period earlier and the PE never stalls on the
current pair's exp -- keeping the HAM clock gate warm (an idle PE re-throttles
2.4->1.2 GHz after ~3.4us). The causal mask multiply runs on DVE in bf16 (2x
mode); LayerNorm's per-token scale/shift is applied via PE rank-1 outer
products (G0 = g (x) rstd, G1 = b (x) 1 + g (x) (-mu*rstd)) and two DVE ops;
head packing into 128-partition tiles uses partition-shifted DVE writes.
"""

import math
import sys
from dataclasses import dataclass

if "/opt/trn_rl_repo" not in sys.path:
    sys.path.insert(0, "/opt/trn_rl_repo")

import numpy as np


@dataclass(frozen=True)
class Cfg:
    B: int = 4
    T: int = 1024
    C: int = 1024
    H: int = 16
    FF: int = 4096

    @property
    def HD(self):
        return self.C // self.H

    @property
    def TQ(self):  # queries per core
        return self.T // 2

    @property
    def NCI(self):  # C / 128 feature tiles
        return self.C // 128

    @property
    def NFF(self):  # FF / 128 hidden tiles
        return self.FF // 128

    @property
    def NKB(self):  # key blocks of 128
        return self.T // 128

    @property
    def BW(self):  # token block width for LN1 phases
        return min(512, self.T)

    @property
    def NTB(self):  # token blocks over all T tokens
        return self.T // self.BW

    def s_kb(self, kb: int) -> int:
        """Start query-column of the computed score region for key block kb.
        Key blocks 0..NKB/2-1 hold this core's own-parity tokens in order;
        NKB/2.. hold the complementary-parity tokens. Causality allows the
        exact suffix [128*kbp, TQ) of queries per block."""
        return 128 * (kb % (self.NKB // 2))

    @property
    def pt_offs(self):
        """Column offsets of each key block's packed score region."""
        offs, o = [], 0
        for kb in range(self.NKB):
            offs.append(o)
            o += self.TQ - self.s_kb(kb)
        return offs + [o]


def build_nc(cfg: Cfg, n_cores: int = 8):
    import concourse.tile as tile
    from concourse import bacc, mybir

    f32 = mybir.dt.float32
    f32r = mybir.dt.float32r
    bf16 = mybir.dt.bfloat16
    Act = mybir.ActivationFunctionType
    Alu = mybir.AluOpType

    C, H, HD, FF = cfg.C, cfg.H, cfg.HD, cfg.FF
    NCI, NFF, NKB, NTB = cfg.NCI, cfg.NFF, cfg.NKB, cfg.NTB
    TQ, T = cfg.TQ, cfg.T
    NP = H // 2  # head pairs
    scale = 1.0 / math.sqrt(HD)
    offs = cfg.pt_offs

    nc = bacc.Bacc(
        "TRN2", target_bir_lowering=False, debug=False, num_devices=n_cores
    )

    # ---- DRAM I/O ----
    xpt = nc.dram_tensor("xpt", [C, T], f32r, kind="ExternalInput")
    msk = nc.dram_tensor("msk", [NKB, 128, 128], bf16, kind="ExternalInput")
    dscr_a = nc.dram_tensor("dscr_a", [H * TQ], bf16, kind="Internal")
    dscr_b = nc.dram_tensor("dscr_b", [H * TQ], bf16, kind="Internal")
    wq = nc.dram_tensor("wq", [C, C], bf16, kind="ExternalInput")
    wk = nc.dram_tensor("wk", [C, C], bf16, kind="ExternalInput")
    wv = nc.dram_tensor("wv", [C, C], bf16, kind="ExternalInput")
    wp = nc.dram_tensor("wp", [C, C], bf16, kind="ExternalInput")
    w1 = nc.dram_tensor("w1", [C, FF], bf16, kind="ExternalInput")
    w2 = nc.dram_tensor("w2", [FF, C], bf16, kind="ExternalInput")
    ln1g = nc.dram_tensor("ln1g", [C], f32r, kind="ExternalInput")
    ln1b = nc.dram_tensor("ln1b", [C], f32r, kind="ExternalInput")
    ln2g = nc.dram_tensor("ln2g", [C], f32r, kind="ExternalInput")
    ln2b = nc.dram_tensor("ln2b", [C], f32r, kind="ExternalInput")
    bpj = nc.dram_tensor("bpj", [C], f32, kind="ExternalInput")
    b1 = nc.dram_tensor("b1", [FF], f32, kind="ExternalInput")
    b2 = nc.dram_tensor("b2", [C], f32, kind="ExternalInput")
    yt = nc.dram_tensor("yt", [C, TQ], f32, kind="ExternalOutput")

    with (
        nc.allow_low_precision(reason="bf16 matmul operands"),
        tile.TileContext(nc) as tc,
    ):
        # ---------------- x DMA first (LN1 starts as soon as possible) ----
        raw, free_raw = tc.tile([128, NCI, T], f32r, name="raw", side="right")
        xpt_r = xpt.rearrange("(ci p) t -> ci p t", p=128)
        for half in range(NTB):
            hsl = slice(half * cfg.BW, (half + 1) * cfg.BW)
            for ci in range(NCI):
                nc.sync.dma_start(out=raw[:, ci, hsl], in_=xpt_r[ci][:, hsl])

        # ---------------- persistent constants / params ----------------
        onesf, free_onesf = tc.tile([128, 512], f32, name="onesf")
        nc.vector.memset(onesf, 1.0)
        ones128, free_ones128 = tc.tile([128, 1], f32r, name="ones128")
        nc.vector.tensor_copy(out=ones128, in_=onesf[:, 0:1])
        ones128b, free_ones128b = tc.tile([128, 1], bf16, name="ones128b")
        nc.vector.tensor_copy(out=ones128b, in_=onesf[:, 0:1])
        onesw, free_onesw = tc.tile([1, cfg.BW], f32r, name="onesw")
        nc.vector.tensor_copy(out=onesw, in_=onesf[0:1, 0 : cfg.BW])
        # lhsT row of ones at partition 64 for the per-head recip broadcast
        oneshi, free_oneshi = tc.tile([65, HD], bf16, name="oneshi")
        nc.vector.tensor_copy(out=oneshi, in_=onesf[0:65, 0:HD])
        epst, free_epst = tc.tile([1, 1], f32, name="epst")
        nc.vector.memset(epst, 1e-5)

        # LN gamma/beta as [1, C] rows (lhsT of the rank-1 broadcasts)
        g1r, free_g1r = tc.tile([1, C], f32r, name="g1r")
        b1r, free_b1r = tc.tile([1, C], f32r, name="b1r")
        g2r, free_g2r = tc.tile([1, C], f32r, name="g2r")
        b2r, free_b2r = tc.tile([1, C], f32r, name="b2r")
        for ptile, v in ((g1r, ln1g), (b1r, ln1b), (g2r, ln2g), (b2r, ln2b)):
            nc.sync.dma_start(
                out=ptile, in_=v.rearrange("(o a) -> o a", o=1)
            )
        bpjt, free_bpjt = tc.tile([128, NCI], f32, name="bpjt")
        nc.sync.dma_start(out=bpjt, in_=bpj.rearrange("(a p) -> p a", p=128))
        b1t, free_b1t = tc.tile([128, NFF], f32, name="b1t")
        nc.sync.dma_start(out=b1t, in_=b1.rearrange("(a p) -> p a", p=128))
        b2t, free_b2t = tc.tile([128, NCI], f32, name="b2t")
        nc.sync.dma_start(out=b2t, in_=b2.rearrange("(a p) -> p a", p=128))
        mskt, free_mskt = tc.tile([128, NKB, 128], bf16, name="mskt")
        nc.sync.dma_start(out=mskt, in_=msk.rearrange("k p m -> p k m"))

        # one PSUM pool for the whole kernel. PSUM is 8 banks of [128, 2KB]:
        # tag "mm" gets 6 rotating slots, tag "av" the other 2 (the AV
        # accumulators live across a whole pair body, so they must not sit
        # in the "mm" rotation or the scores allocs would deadlock on them).
        ps_all = tc.alloc_tile_pool(name="ps_all", bufs=6, space="PSUM")
        wstream = tc.alloc_tile_pool(name="wstream", bufs=8)
        wpair = tc.alloc_tile_pool(name="wpair", bufs=3)

        fill_i = [0]

        def emit_fill(n=2):
            """Dependency-free PE matmuls: keep the HAM clock gate open
            through stretches where real PE work is sparse."""
            with nc.named_scope("fill"):
                pw = ps_all.tile([128, 512], f32, tag="mm", name=f"fw{fill_i[0]}")
                fill_i[0] += 1
                for r in range(n):
                    nc.tensor.matmul(
                        pw, onesf[:, 0:128].bitcast(f32r), onesf.bitcast(f32r),
                        start=(r == 0), stop=(r == n - 1),
                    )

        with nc.named_scope("warmup"):
            for wu in range(6):
                emit_fill(2)

        # x2T = x + attnproj (residual 1), written in the proj phase
        x2t, free_x2t = tc.tile([128, NCI, TQ], bf16, name="x2t")
        # packed normalized heads [128, pair, TQ]
        att2, free_att2 = tc.tile([128, NP, TQ], bf16, name="att2")

        def layernorm(src_ap_fn, dst, g_row, b_row, n_blocks, blk_w, scopename,
                      ones_st=ones128, post_block=None, fills=False):
            """src_ap_fn(ci, sl) -> [128, blk_w] f32r AP; dst [128, NCI, *]
            bf16. Stats via ones-vector matmuls. Apply: per (block, ci) the
            affine per-token transform is built as two PE rank-1 outer
            products G0 = g (x) rstd and G1 = b (x) 1 + g (x) (-mu*rstd),
            then dst = src*G0 + G1 in two DVE ops."""
            with (
                nc.named_scope(scopename),
                tc.tile_pool(name=f"{scopename}_sb", bufs=max(3, n_blocks + 1)) as sbp,
            ):
                stats = []
                for tb in range(n_blocks):
                    sl = slice(tb * blk_w, (tb + 1) * blk_w)
                    psx = ps_all.tile([1, blk_w], f32, tag="mm", name=f"psx{tb}")
                    psq = ps_all.tile([1, blk_w], f32, tag="mm", name=f"psq{tb}")
                    for ci in range(NCI):
                        nc.tensor.matmul(
                            psx, ones_st, src_ap_fn(ci, sl),
                            start=(ci == 0), stop=(ci == NCI - 1),
                        )
                    for ci in range(NCI):
                        x_ap = src_ap_fn(ci, sl)
                        sq = sbp.tile([128, blk_w], bf16, tag="sq", name=f"sq{tb}_{ci}")
                        if ci % 2 == 0:
                            nc.scalar.activation(out=sq, in_=x_ap, func=Act.Square)
                        else:
                            nc.vector.tensor_mul(out=sq, in0=x_ap, in1=x_ap)
                        nc.tensor.matmul(
                            psq, ones128b, sq,
                            start=(ci == 0), stop=(ci == NCI - 1),
                        )
                    stats.append((psx, psq))
                    if fills:
                        emit_fill(2)
                rows = []
                for tb in range(n_blocks):
                    psx, psq = stats[tb]
                    mu = sbp.tile([1, blk_w], f32r, tag="rs", bufs=5, name=f"mu{tb}")
                    nc.scalar.mul(out=mu, in_=psx, mul=1.0 / C)
                    nmu = sbp.tile([1, blk_w], f32r, tag="rs", bufs=5, name=f"nmu{tb}")
                    nc.scalar.mul(out=nmu, in_=mu, mul=-1.0)
                    ms = sbp.tile([1, blk_w], f32r, tag="rs", bufs=5, name=f"ms{tb}")
                    nc.scalar.mul(out=ms, in_=psq, mul=1.0 / C)
                    mu2 = sbp.tile([1, blk_w], f32r, tag="rs", bufs=5, name=f"mu2{tb}")
                    nc.scalar.activation(out=mu2, in_=mu, func=Act.Square)
                    var = sbp.tile([1, blk_w], f32r, tag="rs", bufs=5, name=f"var{tb}")
                    nc.vector.tensor_sub(out=var, in0=ms, in1=mu2)
                    # rstd = exp(-0.5*ln(var+eps)): two fast ACT row ops
                    sd = sbp.tile([1, blk_w], f32r, tag="rs", bufs=5, name=f"sd{tb}")
                    nc.scalar.activation(
                        out=sd, in_=var, func=Act.Ln, bias=epst
                    )
                    c0 = sbp.tile([1, blk_w], f32r, tag=f"c0_{tb}", bufs=1)
                    nc.scalar.activation(
                        out=c0, in_=sd, func=Act.Exp, scale=-0.5
                    )
                    c1 = sbp.tile([1, blk_w], f32r, tag=f"c1_{tb}", bufs=1)
                    nc.vector.tensor_mul(out=c1, in0=nmu, in1=c0)
                    rows.append((c0, c1))
                    if fills:
                        emit_fill(2)
                for tb in range(n_blocks):
                    sl = slice(tb * blk_w, (tb + 1) * blk_w)
                    c0, c1 = rows[tb]
                    for ci in range(NCI):
                        gsl = slice(128 * ci, 128 * (ci + 1))
                        x_ap = src_ap_fn(ci, sl)
                        G0 = ps_all.tile([128, blk_w], f32, tag="mm", name=f"G0_{tb}_{ci}")
                        nc.tensor.matmul(G0, g_row[:, gsl], c0)
                        G1 = ps_all.tile([128, blk_w], f32, tag="mm", name=f"G1_{tb}_{ci}")
                        nc.tensor.matmul(
                            G1, b_row[:, gsl], onesw[:, 0:blk_w],
                            start=True, stop=False,
                        )
                        nc.tensor.matmul(
                            G1, g_row[:, gsl], c1, start=False, stop=True
                        )
                        tmp = sbp.tile([128, blk_w], bf16, tag="tmp", name=f"t{tb}_{ci}")
                        nc.vector.tensor_mul(out=tmp, in0=x_ap, in1=G0)
                        nc.vector.tensor_add(out=dst[:, ci, sl], in0=tmp, in1=G1)
                        if fills and ci % 3 == 2:
                            emit_fill(1)
                    if post_block is not None:
                        post_block(tb)

        # ---------------- attention: per-pair QKV pipelined with softmax ---
        a1, free_a1 = tc.tile([128, NCI, T], bf16, name="a1", side="right")
        # vt: per key block, per head: 64 v-columns + a ones column (fused
        # softmax denominator row in the AV matmul output).
        vt, free_vt = tc.tile([128, NKB, H, HD + 1], bf16, name="vt", side="right")
        for kb in range(NKB):
            nc.vector.tensor_copy(
                out=vt[:, kb, :, HD : HD + 1], in_=onesf[:, 0:H].unsqueeze(2)
            )
        # att holds, per head, O^T rows 0..HD-1 (unnormalized) and the
        # reciprocal softmax denominator in row 64.
        att, free_att = tc.tile([65, H, TQ], bf16, name="att", side="right")

        wq_r = wq.rearrange("(ci p) c -> p ci c", p=128)
        wk_r = wk.rearrange("(ci p) c -> p ci c", p=128)
        wv_r = wv.rearrange("(ci p) c -> p ci c", p=128)

        qk_pool = tc.alloc_tile_pool(name="qk_pool", bufs=3, side="right")
        pt_pool = tc.alloc_tile_pool(name="pt_pool", bufs=4, side="right")

        qts, kts, pts, avps, wvts = {}, {}, {}, {}, {}

        wqts, wkts = {}, {}

        def emit_qk_dma(hp):
            """Prefetch Q/K weight slices for head pair hp."""
            if hp >= NP:
                return
            csl = slice(128 * hp, 128 * (hp + 1))
            wqt = wpair.tile([128, NCI, 128], bf16, tag="wq", bufs=2, name=f"wq{hp}")
            nc.sync.dma_start(out=wqt, in_=wq_r[:, :, csl])
            wkt = wpair.tile([128, NCI, 128], bf16, tag="wk", bufs=2, name=f"wk{hp}")
            nc.sync.dma_start(out=wkt, in_=wk_r[:, :, csl])
            wqts[hp], wkts[hp] = wqt, wkt

        def emit_q(hp):
            """Q projection for head pair hp (feature rows 128*hp..)."""
            if hp >= NP:
                return
            with nc.named_scope("qkv"):
                qt = qk_pool.tile([128, TQ], bf16, tag="qt", name=f"qt{hp}")
                pq = ps_all.tile([128, TQ], f32, tag="mm", name=f"pq{hp}")
                for ci in range(NCI):
                    nc.tensor.matmul(
                        pq, wqts[hp][:, ci, :], a1[:, ci, 0:TQ],
                        start=(ci == 0), stop=(ci == NCI - 1),
                    )
                nc.vector.tensor_copy(out=qt, in_=pq)
                qts[hp] = qt

        def emit_k(hp, tb):
            """K projection for head pair hp, token half tb."""
            if hp >= NP:
                return
            with nc.named_scope("qkv"):
                if tb == 0:
                    kts[hp] = qk_pool.tile([128, T], bf16, tag="kt", name=f"kt{hp}")
                sl = slice(512 * tb, 512 * (tb + 1))
                pk = ps_all.tile([128, 512], f32, tag="mm", name=f"pk{hp}_{tb}")
                for ci in range(NCI):
                    nc.tensor.matmul(
                        pk, wkts[hp][:, ci, :], a1[:, ci, sl],
                        start=(ci == 0), stop=(ci == NCI - 1),
                    )
                nc.vector.tensor_copy(out=kts[hp][:, sl], in_=pk)

        def emit_vdma(g):
            """Prefetch the V weight slice for heads 4g..4g+3."""
            if g >= H // 4:
                return
            csl = slice(256 * g, 256 * (g + 1))
            wvt = wpair.tile([128, NCI, 256], bf16, tag="wv", bufs=2, name=f"wv{g}")
            nc.sync.dma_start(out=wvt, in_=wv_r[:, :, csl])
            wvts[g] = wvt

        def emit_vchunk_kb(g, kb):
            """V projection for heads 4g..4g+3 (pairs 2g, 2g+1), one key
            block. Activations stationary, 256 weight columns moving."""
            if g >= H // 4:
                return
            with nc.named_scope("qkv"):
                kbsl = slice(128 * kb, 128 * (kb + 1))
                pv = ps_all.tile([128, 256], f32, tag="mm", name=f"pv{g}_{kb}")
                for ci in range(NCI):
                    nc.tensor.matmul(
                        pv, a1[:, ci, kbsl], wvts[g][:, ci, :],
                        start=(ci == 0), stop=(ci == NCI - 1),
                    )
                nc.vector.tensor_copy(
                    out=vt[:, kb, 4 * g : 4 * g + 4, 0:HD],
                    in_=pv.rearrange("p (h d) -> p h d", h=4),
                )

        def emit_scores_kb(hp, kb):
            """Scores + exp + causal mask for both heads of pair hp, one
            key block."""
            if hp >= NP:
                return
            with nc.named_scope("attn"):
                if kb == 0:
                    p0 = pt_pool.tile([128, offs[-1]], bf16, tag="pt", name=f"pt{2 * hp}")
                    p1 = pt_pool.tile([128, offs[-1]], bf16, tag="pt", name=f"pt{2 * hp + 1}")
                    pts[hp] = (p0, p1)
                qt, kt = qts[hp], kts[hp]
                s = cfg.s_kb(kb)
                n = TQ - s
                kbsl = slice(128 * kb, 128 * (kb + 1))
                pss = []
                for idx in range(2):
                    po = idx * HD
                    ps_s = ps_all.tile([128, 512], f32, tag="mm", name=f"sc{hp}_{kb}_{idx}")
                    nc.tensor.matmul(
                        ps_s[:, 0:n],
                        kt[po : po + HD, kbsl],
                        qt[po : po + HD, s:TQ],
                    )
                    pss.append(ps_s)
                for idx in range(2):
                    dst = pts[hp][idx]
                    nc.scalar.activation(
                        out=dst[:, offs[kb] : offs[kb] + n],
                        in_=pss[idx][:, 0:n],
                        func=Act.Exp, scale=scale,
                    )
                    # causal mask: first 128 columns of the block's region
                    # are the partially-visible diagonal zone. bf16 -> DVE 2x.
                    nc.vector.tensor_mul(
                        out=dst[:, offs[kb] : offs[kb] + 128],
                        in0=dst[:, offs[kb] : offs[kb] + 128],
                        in1=mskt[:, kb, :],
                    )

        def emit_av_kb(hp, kb):
            """One key block of the AV accumulation for both heads of pair
            hp (inputs were produced one pair-period earlier)."""
            if hp < 0:
                return
            with nc.named_scope("attn"):
                s = cfg.s_kb(kb)
                for idx in range(2):
                    h = 2 * hp + idx
                    if kb == 0:
                        avps[h] = ps_all.tile(
                            [65, TQ], f32, tag="av", bufs=2, name=f"av{h}"
                        )
                    nc.tensor.matmul(
                        avps[h][:, s:TQ],
                        vt[:, kb, h, :],
                        pts[hp][idx][:, offs[kb] : offs[kb + 1]],
                        start=(kb == 0), stop=(kb == NKB - 1),
                        skip_group_check=True,
                    )

        def emit_av_finish(hp):
            """Copy unnormalized O^T and denominator rows out of PSUM."""
            with nc.named_scope("attn"):
                for idx in range(2):
                    h = 2 * hp + idx
                    if hp >= 6:
                        nc.scalar.copy(out=att[0:64, h, :], in_=avps[h][0:64, :])
                    else:
                        nc.vector.tensor_copy(
                            out=att[0:64, h, :], in_=avps[h][0:64, :]
                        )
                    nc.vector.tensor_copy(
                        out=att[64:65, h, :], in_=avps[h][64:65, :]
                    )

        def emit_recip(hs, nh):
            """Batch-reciprocal the denominator rows of heads hs..hs+nh-1
            via a DRAM round-trip spreading them over 128 partitions."""
            assert (nh * TQ) % 128 == 0
            with nc.named_scope("attn"):
                hsl = slice(hs, hs + nh)
                fl = nh * TQ // 128
                nc.sync.dma_start(
                    out=dscr_a.rearrange("(o h t) -> o h t", o=1, h=H)[:, hsl, :],
                    in_=att[64:65, hsl, :],
                )
                dwide = pt_pool.tile([128, fl], bf16, tag="dw", bufs=2, name=f"dw{hs}")
                nc.sync.dma_start(
                    out=dwide,
                    in_=dscr_a[hs * TQ : (hs + nh) * TQ].rearrange(
                        "(p f) -> p f", p=128
                    ),
                )
                nc.vector.reciprocal(out=dwide, in_=dwide)
                nc.sync.dma_start(
                    out=dscr_b[hs * TQ : (hs + nh) * TQ].rearrange(
                        "(p f) -> p f", p=128
                    ),
                    in_=dwide,
                )
                nc.sync.dma_start(
                    out=att[64:65, hsl, :],
                    in_=dscr_b.rearrange("(o h t) -> o h t", o=1, h=H)[:, hsl, :],
                )

        def emit_norm(hp):
            """Normalize pair hp's heads by the reciprocal denominators and
            pack them into att2[:, hp, :] (odd head via partition-shifted
            DVE write to partitions 64..127)."""
            with nc.named_scope("attn"):
                for idx in range(2):
                    h = 2 * hp + idx
                    bc = ps_all.tile([64, TQ], f32, tag="mm", name=f"bc{h}")
                    nc.tensor.matmul(bc, oneshi[64:65, :], att[64:65, h, :])
                    psl = slice(64 * idx, 64 * idx + 64)
                    nc.vector.tensor_mul(
                        out=att2[psl, hp, :], in0=att[0:64, h, :], in1=bc
                    )

        # pipeline: scores of pair hp+1, AV of pair hp, V of pair hp+1,
        # and Q/K of pair hp+2 are interleaved at key-block granularity, so
        # every PE matmul's inputs are one pair-period old and the PE
        # streams continuously (keeps the HAM clock gate warm).
        def ln1_post_block(tb):
            """Interleave the first pair's projections/scores with LN1's
            DVE-bound apply phase: block 0 gates queries + the first key
            half, block 1 the rest."""
            if tb == 0:
                emit_qk_dma(0)
                emit_qk_dma(1)
                emit_vdma(0)
                emit_q(0)
                emit_k(0, 0)
                for kb in range(NKB // 2):
                    emit_scores_kb(0, kb)
                    emit_vchunk_kb(0, kb)
            else:
                emit_k(0, 1)
                for kb in range(NKB // 2, NKB):
                    emit_scores_kb(0, kb)
                    emit_vchunk_kb(0, kb)
                emit_q(1)
                emit_k(1, 0)
                emit_k(1, 1)

        # ---------------- LN1 over all T tokens ----------------
        layernorm(
            lambda ci, sl: raw[:, ci, sl], a1, g1r, b1r, NTB, cfg.BW, "ln1",
            post_block=ln1_post_block, fills=True,
        )
        for hp in range(NP):
            g = (hp + 1) // 2
            vchunk = hp % 2 == 1 and g < H // 4
            if vchunk:
                emit_vdma(g)
            for kb in range(NKB):
                emit_scores_kb(hp + 1, kb)
                emit_av_kb(hp, kb)
                if vchunk:
                    emit_vchunk_kb(g, kb)
                if hp >= 6 and kb % 2 == 0:
                    emit_fill(2)
                if kb == 1:
                    emit_qk_dma(hp + 2)
                elif kb == 3:
                    emit_q(hp + 2)
                elif kb == 5:
                    emit_k(hp + 2, 0)
                elif kb == 7:
                    emit_k(hp + 2, 1)
            emit_av_finish(hp)
            emit_recip(2 * hp, 2)
            if hp >= 1:
                # previous pair's reciprocals are back from DRAM by now
                emit_norm(hp - 1)
        emit_fill(4)
        emit_norm(7)
        emit_fill(2)

        # ---------------- attention out-proj + residual 1 ----------------
        with nc.named_scope("proj"):
            pp = [
                ps_all.tile(
                    [128, TQ], f32,
                    tag=("mm" if i < 6 else "av"),
                    bufs=(6 if i < 6 else 2),
                    name=f"pp{i}",
                )
                for i in range(NCI)
            ]
            for ci in range(NCI):
                wt = wstream.tile([128, C], bf16, tag="w", name=f"wpt{ci}")
                nc.sync.dma_start(out=wt, in_=wp[128 * ci : 128 * (ci + 1)])
                for co in range(NCI):
                    nc.tensor.matmul(
                        pp[co],
                        wt[:, 128 * co : 128 * (co + 1)],
                        att2[:, ci, :],
                        start=(ci == 0), stop=(ci == NCI - 1),
                    )
            for co in range(NCI):
                nc.vector.scalar_tensor_tensor(
                    out=x2t[:, co, :],
                    in0=pp[co],
                    scalar=bpjt[:, co : co + 1],
                    in1=raw[:, co, 0:TQ],
                    op0=Alu.add,
                    op1=Alu.add,
                )
        pt_pool.release()
        qk_pool.release()
        free_att()
        free_vt()
        free_a1()
        free_raw()

        # ---------------- LN2 ----------------
        a2, free_a2 = tc.tile([128, NCI, TQ], bf16, name="a2", side="right")
        layernorm(
            lambda ci, sl: x2t[:, ci, sl], a2, g2r, b2r, 1, TQ, "ln2",
            ones_st=ones128b, fills=True,
        )

        # ---------------- FFN ----------------
        hsb, free_hsb = tc.tile([128, NFF, TQ], bf16, name="hsb", side="right")
        with nc.named_scope("ffn1"):
            for cog in range(NFF // 8):
                pf = [
                    ps_all.tile(
                        [128, TQ], f32,
                        tag=("mm" if i < 6 else "av"),
                        bufs=(6 if i < 6 else 2),
                        name=f"pf{cog}_{i}",
                    )
                    for i in range(8)
                ]
                for ci in range(NCI):
                    wt = wstream.tile([128, 1024], bf16, tag="w", name=f"w1t{cog}_{ci}")
                    nc.sync.dma_start(
                        out=wt,
                        in_=w1[
                            128 * ci : 128 * (ci + 1),
                            1024 * cog : 1024 * (cog + 1),
                        ],
                    )
                    for co in range(8):
                        nc.tensor.matmul(
                            pf[co],
                            wt[:, 128 * co : 128 * (co + 1)],
                            a2[:, ci, :],
                            start=(ci == 0), stop=(ci == NCI - 1),
                        )
                for co in range(8):
                    hco = cog * 8 + co
                    nc.scalar.activation(
                        out=hsb[:, hco, :],
                        in_=pf[co],
                        func=Act.Gelu,
                        bias=b1t[:, hco : hco + 1],
                    )

        yts, free_yts = tc.tile([128, NCI, TQ], f32, name="yts", side="right")
        with nc.named_scope("ffn2"):
            py = [
                ps_all.tile(
                    [128, TQ], f32,
                    tag=("mm" if i < 6 else "av"),
                    bufs=(6 if i < 6 else 2),
                    name=f"py{i}",
                )
                for i in range(NCI)
            ]
            for fi in range(NFF):
                wt = wstream.tile([128, C], bf16, tag="w", name=f"w2t{fi}")
                nc.sync.dma_start(out=wt, in_=w2[128 * fi : 128 * (fi + 1)])
                for co in range(NCI):
                    nc.tensor.matmul(
                        py[co],
                        wt[:, 128 * co : 128 * (co + 1)],
                        hsb[:, fi, :],
                        start=(fi == 0), stop=(fi == NFF - 1),
                    )
            yt_r = yt.rearrange("(ci p) t -> ci p t", p=128)
            for co in range(NCI):
                nc.vector.scalar_tensor_tensor(
                    out=yts[:, co, :],
                    in0=py[co],
                    scalar=b2t[:, co : co + 1],
                    in1=x2t[:, co, :],
                    op0=Alu.add,
                    op1=Alu.add,
                )
                nc.sync.dma_start(out=yt_r[co], in_=yts[:, co, :])
        free_yts()
        free_hsb()
        free_a2()
        free_att2()
        free_x2t()
        wpair.release()
        wstream.release()
        ps_all.release()
        free_mskt()
        free_b2t()
        free_b1t()
        free_bpjt()
        free_b2r()
        free_g2r()
        free_b1r()
        free_g1r()
        free_epst()
        free_oneshi()
        free_onesw()
        free_ones128b()
        free_ones128()
        free_onesf()

    nc.compile()
    return nc


def prep_core_inputs(cfg: Cfg, inputs: dict, b: int, j: int) -> dict:
    """Host-side slicing/permutation for core (batch b, parity j)."""
    T, TQ, NKB = cfg.T, cfg.TQ, cfg.NKB
    x = np.asarray(inputs["x"])
    perm = np.concatenate([np.arange(j, T, 2), np.arange(1 - j, T, 2)])
    xp = x[b][perm]  # [T, C]
    xpt = np.ascontiguousarray(xp.T, dtype=np.float32)

    import ml_dtypes

    qtok = perm[:TQ]
    ktok = perm
    mask = np.ones((NKB, 128, 128), dtype=np.float32)
    for kb in range(NKB):
        s = cfg.s_kb(kb)
        kt = ktok[128 * kb : 128 * (kb + 1)]  # [128]
        qt = qtok[s : s + 128]  # [128]
        allowed = qt[None, :] >= kt[:, None]  # [128, 128]
        mask[kb] = np.where(allowed, 1.0, 0.0)
    return {"xpt": xpt, "msk": mask.astype(ml_dtypes.bfloat16)}


def prep_shared_inputs(cfg: Cfg, inputs: dict) -> dict:
    import ml_dtypes

    C = cfg.C
    f32 = np.float32
    bf16 = ml_dtypes.bfloat16

    def wq2d(w):  # [H, C, HD] -> [C, H*HD]
        w = np.asarray(w)
        return np.ascontiguousarray(
            w.transpose(1, 0, 2).reshape(C, C)
        ).astype(bf16)

    return {
        "wq": wq2d(inputs["Wq"]),
        "wk": wq2d(inputs["Wk"]),
        "wv": wq2d(inputs["Wv"]),
        "wp": np.ascontiguousarray(inputs["Wproj"]).astype(bf16),
        "w1": np.ascontiguousarray(inputs["W1"]).astype(bf16),
        "w2": np.ascontiguousarray(inputs["W2"]).astype(bf16),
        "ln1g": np.ascontiguousarray(inputs["ln1_g"], dtype=f32),
        "ln1b": np.ascontiguousarray(inputs["ln1_b"], dtype=f32),
        "ln2g": np.ascontiguousarray(inputs["ln2_g"], dtype=f32),
        "ln2b": np.ascontiguousarray(inputs["ln2_b"], dtype=f32),
        "bpj": np.ascontiguousarray(inputs["bproj"], dtype=f32),
        "b1": np.ascontiguousarray(inputs["b1"], dtype=f32),
        "b2": np.ascontiguousarray(inputs["b2"], dtype=f32),
    }


def run(cfg: Cfg, inputs: dict, n_cores: int = 8, trace: bool = False):
    from concourse.bass_utils import run_bass_kernel_spmd

    nc = build_nc(cfg, n_cores=n_cores)
    shared = prep_shared_inputs(cfg, inputs)
    in_maps = []
    cores = []
    for core in range(n_cores):
        b, j = divmod(core, 2)
        b = b % cfg.B
        in_maps.append({**prep_core_inputs(cfg, inputs, b, j), **shared})
        cores.append((b, j))
    res = run_bass_kernel_spmd(
        nc, in_maps, core_ids=list(range(n_cores)), trace=trace
    )
    out = np.zeros((cfg.B, cfg.T, cfg.C), dtype=np.float32)
    for core, (b, j) in enumerate(cores):
        ytv = res.results[core]["yt"]  # [C, TQ]
        perm = np.concatenate(
            [np.arange(j, cfg.T, 2), np.arange(1 - j, cfg.T, 2)]
        )
        out[b, perm[: cfg.TQ], :] = ytv.T
    return out, res


def kernel(**inputs) -> np.ndarray:
    out, _ = run(Cfg(), inputs, n_cores=8, trace=False)
    return out


if __name__ == "__main__":
    # quick self-exercise at full size with random data
    rng = np.random.default_rng(0)
    cfg = Cfg()
    ins = {
        "x": rng.standard_normal((cfg.B, cfg.T, cfg.C)).astype(np.float32),
        "ln1_g": np.ones(cfg.C, np.float32),
        "ln1_b": np.zeros(cfg.C, np.float32),
        "ln2_g": np.ones(cfg.C, np.float32),
        "ln2_b": np.zeros(cfg.C, np.float32),
        "Wq": rng.standard_normal((cfg.H, cfg.C, cfg.HD)).astype(np.float32)
        * 0.02,
        "Wk": rng.standard_normal((cfg.H, cfg.C, cfg.HD)).astype(np.float32)
        * 0.02,
        "Wv": rng.standard_normal((cfg.H, cfg.C, cfg.HD)).astype(np.float32)
        * 0.02,
        "Wproj": rng.standard_normal((cfg.C, cfg.C)).astype(np.float32) * 0.02,
        "bproj": np.zeros(cfg.C, np.float32),
        "W1": rng.standard_normal((cfg.C, cfg.FF)).astype(np.float32) * 0.02,
        "b1": np.zeros(cfg.FF, np.float32),
        "W2": rng.standard_normal((cfg.FF, cfg.C)).astype(np.float32) * 0.02,
        "b2": np.zeros(cfg.C, np.float32),
    }
    y = kernel(**ins)
    print("ran, out", y.shape, y.dtype, float(np.abs(y).max()))
